# revision 11
# baseline (speedup 1.0000x reference)
"""3-layer GAT on Trainium2, 8 NeuronCores — v2.

Strategy (dst-sharded, replicated tables):
- Nodes are remapped into 8 slices of 6656 rows (6250 real + pad); each core
  owns one slice of destination nodes and all edges pointing into it.
- Per layer, every core builds its slice of a node table with 512-byte rows
  [h(256 fp8) | el(8 f32 = 32B) | pad]; an AllGather (Shared outputs)
  replicates the full 53248-row table to every core. er values for the
  core's own dst nodes stay in SBUF (erstore), never in the table.
- Edge phase per layer runs in TWO passes over the 49 dst windows: pass LOW
  aggregates the edges whose src lives in the first table half, stashing
  partial sums in SBUF; pass HIGH adds the rest, normalizes (softmax
  denominator after aggregation: alpha = ex/sum(ex) is scale invariant,
  |logits| < 10 so no segment-max) and runs the node phase. The half-0
  AllGather of the next layer's table (fired mid HIGH pass) then overlaps
  the next layer's LOW pass.
- Per (window, half): one dma_gather with exact (16-padded) edge counts;
  selT one-hots are generated on-chip (iota + is_equal against the dst-lane
  stream); selS (dst-major, needed to expand er to edges) streams from HBM;
  e = el_src + er_dst, leaky-relu + exp run edge-wise; one-hot matmuls
  segment-sum ex*h and ex into PSUM per dst.
- Layer 3 commutes the output projection with aggregation:
  sum(ex*h2) @ Wo == sum(ex*(h2@Wo)), with el3 = h2 @ (Wo@alo^T).
"""
import numpy as np
import ml_dtypes
from contextlib import ExitStack

import concourse.bass as bass
import concourse.tile as tile
from concourse import bacc, mybir
from concourse.bass_utils import run_bass_kernel_spmd
from concourse.masks import make_identity

BF16 = ml_dtypes.bfloat16
FP8 = ml_dtypes.float8_e4m3

N_NODES = 50000
IN_F = 128
N_CLASSES = 40
CORES = 8
SPR = 6250          # real dst nodes per core
SP = 6656           # slice rows per core (52 * 128)
NT = SP * CORES     # 53248 padded table rows
W = 49              # dst windows per core (ceil(6250/128))
ROWB = 512          # table row bytes (fp8 elems)
NTILE = SP // 128   # 52 node tiles per slice
EXPF = mybir.ActivationFunctionType.Exp
PRELU = mybir.ActivationFunctionType.Prelu
RELUF = mybir.ActivationFunctionType.Relu
COPYF = mybir.ActivationFunctionType.Copy

HT = NTILE // 2      # 26 tiles per collective half
HROWS = HT * 128     # 3328 slice rows per half
HBLK = HROWS * CORES # 26624 table rows per half; also the gather split
                     # boundary (aligns gather deps with one AllGather, and
                     # 26624 < 32768 keeps int16 indices valid)


def _remap(n):
    """Global table row for node n, laid out (half, core, tile, row) so that
    half-wise partial AllGathers are contiguous in both the slice and the
    full table (AllGather concatenates per-core inputs)."""
    c = n // SPR
    r = n % SPR
    t = r // 128
    h = t // HT
    return h * HBLK + c * HROWS + (t % HT) * 128 + (r % 128)


def _wrap16(vals):
    """dma_gather index layout: flat idx i -> [i%16, i//16], replicated to
    all 8 groups of 16 partitions. len(vals) must be a multiple of 16."""
    n = len(vals)
    blk = np.zeros((16, n // 16), np.int16)
    blk[np.arange(n) % 16, np.arange(n) // 16] = vals
    return np.tile(blk, (8, 1))


def _host_prep(src, dst):
    """Group edges by (dst core, dst window, table half). Returns per-core
    lists of (gather idx, dst lane) arrays indexed [W][2], plus the
    32-padded counts (32 keeps the chunk-tail memset partition-aligned)."""
    gsrc = _remap(src.astype(np.int64))
    d64 = dst.astype(np.int64)
    core = d64 // SPR
    ld = d64 % SPR
    w = ld >> 7
    dstl = (ld & 127).astype(np.int64)
    hi = (gsrc >= HBLK).astype(np.int64)

    key = (core * W + w) * 2 + hi
    order = np.argsort(key, kind="stable")
    gsrc_s, dstl_s, key_s = gsrc[order], dstl[order], key[order]

    per_core = []
    counts = np.zeros((CORES, W, 2), np.int64)
    for c in range(CORES):
        ed = [[None, None] for _ in range(W)]
        for wi in range(W):
            for s in (0, 1):
                k = (c * W + wi) * 2 + s
                lo = np.searchsorted(key_s, k)
                hiend = np.searchsorted(key_s, k + 1)
                g = (gsrc_s[lo:hiend] - (HBLK if s else 0)).astype(np.int16)
                dl = dstl_s[lo:hiend].astype(np.int16)
                ed[wi][s] = (g, dl)
                counts[c, wi, s] = max(((len(g) + 31) // 32) * 32, 32)
        per_core.append(ed)
    return per_core, counts


def _alar_block(al, ar, fout):
    """[fout, 16]: col j (<8) extracts el head j, col j+8 er head j."""
    H, F = al.shape
    m = np.zeros((fout, 16), np.float32)
    for j in range(H):
        m[j * F:(j + 1) * F, j] = al[j]
        m[j * F:(j + 1) * F, j + 8] = ar[j]
    return m


def _layout(n16s):
    """Shared program/data layout derived from the per-(window,half) counts
    (max over cores). Returns chunk counts and stream offsets."""
    CW = [[int((n16s[wi][s] + 127) // 128) for s in (0, 1)] for wi in range(W)]
    CMAX = max(max(r) for r in CW)
    off = [np.concatenate([[0], np.cumsum([int(n16s[wi][s]) // 16 for wi in range(W)])])
           for s in (0, 1)]
    soff = [np.concatenate([[0], np.cumsum([CW[wi][s] * 128 for wi in range(W)])])
            for s in (0, 1)]
    coff = [np.concatenate([[0], np.cumsum([CW[wi][s] for wi in range(W)])])
            for s in (0, 1)]
    return CW, CMAX, off, soff, coff


def _build_program(n16s):
    """n16s: [W][2] 16-padded edge counts (max over cores; all cores run
    identical shapes, shorter cores pad with idx 0 / lane 400)."""
    CW, CMAX, off, soff, coff = _layout(n16s)

    nc = bacc.Bacc("TRN2", target_bir_lowering=False, debug=False, num_devices=CORES)
    f32, bf16, i16 = mybir.dt.float32, mybir.dt.bfloat16, mybir.dt.int16
    fp8 = mybir.dt.float8e4

    xsl = nc.declare_dram_parameter("xsl", [SP, IN_F], f32, isOutput=False)
    gidx_d = [nc.declare_dram_parameter(f"gidx{s}", [128, int(off[s][-1])], i16,
                                        isOutput=False) for s in (0, 1)]
    selS_d = [nc.declare_dram_parameter(f"selS{s}", [128, int(soff[s][-1])], bf16,
                                        isOutput=False) for s in (0, 1)]
    dcol_d = [nc.declare_dram_parameter(f"dcol{s}", [128, int(coff[s][-1])], bf16,
                                        isOutput=False) for s in (0, 1)]
    mneg_d = nc.declare_dram_parameter("mneg", [1, SP], bf16, isOutput=False)
    w1_d = nc.declare_dram_parameter("w1", [IN_F, 256], bf16, isOutput=False)
    w2_d = nc.declare_dram_parameter("w2", [128, 2, 256], bf16, isOutput=False)
    wo_d = nc.declare_dram_parameter("wo", [128, 2, N_CLASSES], bf16, isOutput=False)
    alar_d = nc.declare_dram_parameter("alar", [128, 3, 2, 16], bf16, isOutput=False)
    outy = nc.declare_dram_parameter("outy", [SP, N_CLASSES], f32, isOutput=True)

    with ExitStack() as ctx:
        tc = ctx.enter_context(tile.TileContext(nc))
        const = ctx.enter_context(tc.tile_pool(name="const", bufs=1))
        dram = ctx.enter_context(tc.tile_pool(name="dram", bufs=1, space="DRAM"))
        gpool = ctx.enter_context(tc.tile_pool(name="gpool", bufs=3))
        spool = ctx.enter_context(tc.tile_pool(name="spool", bufs=3))
        npool = ctx.enter_context(tc.tile_pool(name="npool", bufs=2))
        pwin = ctx.enter_context(tc.tile_pool(name="pwin", bufs=2, space="PSUM"))
        pnode = ctx.enter_context(tc.tile_pool(name="pnode", bufs=2, space="PSUM"))

        Tsl_h = [dram.tile([HROWS, ROWB], fp8, name="tsl0"),
                 dram.tile([HROWS, ROWB], fp8, name="tsl1")]
        # one tile per (layer, half); addr_space comes from USE_SHARED
        import os as _os
        _aspace = "Shared" if _os.environ.get("GAT_SHARED", "1") == "1" else "Local"
        Tfull = [[dram.tile([HBLK, ROWB], fp8, addr_space=_aspace,
                            name=f"tf{l}{s}") for s in (0, 1)] for l in range(3)]

        def Tslice_rows(r0, r1):
            h = r0 // HROWS
            assert (r1 - 1) // HROWS == h
            return Tsl_h[h][r0 - h * HROWS:r1 - h * HROWS, :]

        gidx_t = []
        for s in (0, 1):
            t_ = const.tile([128, int(off[s][-1])], i16, name=f"gidx{s}t")
            nc.sync.dma_start(out=t_[:], in_=gidx_d[s][:, :])
            gidx_t.append(t_)
        dcol_t = []
        for s in (0, 1):
            t_ = const.tile([128, int(coff[s][-1])], bf16, name=f"dcol{s}t")
            nc.sync.dma_start(out=t_[:], in_=dcol_d[s][:, :])
            dcol_t.append(t_)
        mneg_t = const.tile([1, SP], bf16)
        nc.sync.dma_start(out=mneg_t[:], in_=mneg_d[:, :])
        w1_t = const.tile([IN_F, 256], bf16)
        nc.sync.dma_start(out=w1_t[:], in_=w1_d[:, :])
        w2_t = const.tile([128, 2, 256], bf16)
        nc.sync.dma_start(out=w2_t[:], in_=w2_d[:, :, :])
        wo_t = const.tile([128, 2, N_CLASSES], bf16)
        nc.sync.dma_start(out=wo_t[:], in_=wo_d[:, :, :])
        alar_t = const.tile([128, 3, 2, 16], bf16)
        nc.sync.dma_start(out=alar_t[:], in_=alar_d[:, :, :, :])
        ident = const.tile([128, 128], bf16)
        make_identity(nc, ident[:])
        ones16 = const.tile([1, 16], bf16)
        nc.vector.memset(ones16[:], 1.0)

        # iota grid for on-chip selT generation: iota_d[p, c, d] = d
        iota_d = const.tile([128, CMAX, 128], bf16)
        nc.gpsimd.iota(iota_d[:], pattern=[[0, CMAX], [1, 128]], base=0,
                       channel_multiplier=0, allow_small_or_imprecise_dtypes=True)

        # persistent per-layer stores
        erstore = [const.tile([128, NTILE, 8], bf16, name=f"ers{i}") for i in range(2)]
        stash = const.tile([128, W, 264], bf16)

        def emit_table_rows(l_next, h_T, h_node_src, w):
            """Assemble table row tile [h fp8 | el f32] for node rows
            [w*128,(w+1)*128) of layer l_next's table and DMA to Tslice.
            h_T: [128,2,128] bf16 feature-major; h_node_src: node-major
            h values [128, 256] (any float dtype)."""
            row_t = npool.tile([128, ROWB], fp8, tag="row")
            nc.vector.tensor_copy(out=row_t[:, 0:256], in_=h_node_src)
            el_ps = pnode.tile([16, 128], f32, tag="nps")
            for kt in range(2):
                nc.tensor.matmul(out=el_ps[:], lhsT=alar_t[:, l_next, kt, :],
                                 rhs=h_T[:, kt, :], start=(kt == 0), stop=False)
            nc.tensor.matmul(out=el_ps[:], lhsT=ones16[:],
                             rhs=mneg_t[:, w * 128:(w + 1) * 128], start=False, stop=True)
            elb = npool.tile([128, 128], bf16, tag="elb")
            nc.scalar.activation(out=elb[0:16, :], in_=el_ps[:], func=COPYF)
            elT_ps = pnode.tile([128, 128], bf16, tag="nps")
            nc.tensor.transpose(out=elT_ps[:], in_=elb[:], identity=ident[:])
            nc.scalar.activation(out=row_t[:, 256:288].bitcast(f32), in_=elT_ps[:, 0:8],
                                 func=COPYF)
            nc.scalar.activation(out=erstore[l_next % 2][:, w, :], in_=elT_ps[:, 8:16],
                                 func=COPYF)
            nc.sync.dma_start(out=Tslice_rows(w * 128, (w + 1) * 128), in_=row_t[:])

        # ---- P0: layer-1 table from x ----
        for t in range(NTILE):
            x_t = npool.tile([128, IN_F], f32, tag="xt")
            nc.sync.dma_start(out=x_t[:], in_=xsl[t * 128:(t + 1) * 128, :])
            xb = npool.tile([128, IN_F], bf16, tag="xb")
            nc.scalar.activation(out=xb[:], in_=x_t[:], func=COPYF)
            xT_ps = pnode.tile([128, 128], bf16, tag="nps")
            nc.tensor.transpose(out=xT_ps[:], in_=xb[:], identity=ident[:])
            xT = npool.tile([128, 128], bf16, tag="xT")
            nc.scalar.activation(out=xT[:], in_=xT_ps[:], func=COPYF)
            h_ps = pnode.tile([128, 2, 128], f32, tag="nps")
            for mt in range(2):
                nc.tensor.matmul(out=h_ps[:, mt, :], lhsT=w1_t[:, mt * 128:(mt + 1) * 128],
                                 rhs=xT[:], start=True, stop=True)
            h_T = npool.tile([128, 2, 128], bf16, tag="hT")
            nc.scalar.activation(out=h_T[:], in_=h_ps[:], func=COPYF)
            hb_ps = pnode.tile([128, 2, 128], bf16, tag="nps")
            for t2 in range(2):
                nc.tensor.transpose(out=hb_ps[:, t2, :], in_=h_T[:, t2, :], identity=ident[:])
            hb = npool.tile([128, 256], f32, tag="hb")
            nc.scalar.activation(out=hb[:], in_=hb_ps[:].rearrange("p a b -> p (a b)"),
                                 func=COPYF)
            emit_table_rows(0, h_T, hb[:], t)
            if t in (HT - 1, NTILE - 1):
                h = 0 if t == HT - 1 else 1
                nc.gpsimd.collective_compute(
                    "AllGather", mybir.AluOpType.bypass,
                    replica_groups=[list(range(CORES))],
                    ins=[Tsl_h[h].opt()],
                    outs=[Tfull[0][h][:, :]])

        # ---- 3 layers, each as LOW pass then HIGH pass over windows ----
        def window_half(l, wi, s):
            """Gather + aggregate one (window, half); returns agg PSUM tile."""
            C = CW[wi][s]
            n16 = int(n16s[wi][s])
            goff = int(off[s][wi])
            gwid = n16 // 16
            sof = int(soff[s][wi])
            cof = int(coff[s][wi])

            g_win = gpool.tile([128, C, ROWB], fp8, tag="gwin")
            nc.gpsimd.dma_gather(
                out_ap=g_win[:], in_ap=Tfull[l][s][:, :],
                idxs_ap=gidx_t[s][:, goff:goff + gwid],
                num_idxs=n16, num_idxs_reg=n16, elem_size=ROWB,
                single_packet=False)
            selS = spool.tile([128, C, 128], bf16, tag="selS")
            nc.sync.dma_start(out=selS[:].rearrange("p c d -> p (c d)"),
                              in_=selS_d[s][:, sof:sof + C * 128])
            selT = spool.tile([128, C, 128], bf16, tag="selT")
            nc.vector.tensor_tensor(
                out=selT[:], in0=iota_d[:, 0:C, :],
                in1=dcol_t[s][:, cof:cof + C].rearrange("p (c o) -> p c o", o=1)
                    .to_broadcast([128, C, 128]),
                op=mybir.AluOpType.is_equal)

            # er expansion to edges: pp[slot, 8]
            pp_ps = pwin.tile([128, C, 8], f32, tag="ppps", bufs=3)
            for c in range(C):
                nc.tensor.matmul(out=pp_ps[:, c, :], lhsT=selS[:, c, :],
                                 rhs=erstore[l % 2][:, wi, :], start=True, stop=True)
            # e = el + er ; leaky-relu (parametric_relu table) ; ex = exp(e)
            e_t = spool.tile([128, C, 8], f32, tag="et")
            nc.vector.tensor_tensor(
                out=e_t[:], in0=g_win[:, :, 256:288].bitcast(f32),
                in1=pp_ps[:], op=mybir.AluOpType.add)
            lk_t = spool.tile([128, C, 8], f32, tag="lkt")
            nc.scalar.activation(out=lk_t[:], in_=e_t[:], func=PRELU, alpha=0.2)
            ex_t = spool.tile([128, C, 8], bf16, tag="ext")
            nc.scalar.activation(out=ex_t[:], in_=lk_t[:], func=EXPF)

            rhs_w = spool.tile([128, C, 264], bf16, tag="rhsw")
            if l < 2:
                nc.vector.tensor_tensor(
                    out=rhs_w[:, :, 0:256].rearrange("p c (h f) -> p c h f", h=8),
                    in0=g_win[:, :, 0:256].rearrange("p c (h f) -> p c h f", h=8),
                    in1=ex_t[:].rearrange("p c (h o) -> p c h o", o=1)
                        .to_broadcast([128, C, 8, 32]),
                    op=mybir.AluOpType.mult)
            else:
                nc.vector.tensor_tensor(
                    out=rhs_w[:, :, 0:256],
                    in0=g_win[:, :, 0:256],
                    in1=ex_t[:, :, 0:1].to_broadcast([128, C, 256]),
                    op=mybir.AluOpType.mult)
            nc.scalar.activation(out=rhs_w[:, :, 256:264], in_=ex_t[:], func=COPYF)
            # chunk-tail slots (no gather, stale SBUF) could be NaN; zero
            # them in 32-partition pieces (engine APs with base partition
            # > 0 are limited to one 32-partition group)
            lane0 = n16 - (C - 1) * 128
            for b in range(lane0, 128, 32):
                nc.vector.memset(rhs_w[b:b + 32, C - 1, :], 0.0)

            agg_ps = pwin.tile([128, 264], f32, tag="aggps")
            for c in range(C):
                nc.tensor.matmul(out=agg_ps[:], lhsT=selT[:, c, :], rhs=rhs_w[:, c, :],
                                 start=(c == 0), stop=(c == C - 1))
            return agg_ps

        for l in range(3):
            # LOW pass
            for wi in range(W):
                agg_ps = window_half(l, wi, 0)
                nc.scalar.activation(out=stash[:, wi, :], in_=agg_ps[:], func=COPYF)
            # HIGH pass
            for wi in range(W):
                agg_ps = window_half(l, wi, 1)
                u_f = npool.tile([128, 264], f32, tag="uf")
                nc.vector.tensor_tensor(out=u_f[:], in0=agg_ps[:], in1=stash[:, wi, :],
                                        op=mybir.AluOpType.add)
                if l < 2:
                    s_rec = npool.tile([128, 8], f32, tag="srec")
                    nc.vector.tensor_scalar_add(s_rec[:], u_f[:, 256:264], 1e-16)
                    nc.vector.reciprocal(out=s_rec[:], in_=s_rec[:])
                    u_t = npool.tile([128, 256], f32, tag="ut")
                    nc.vector.tensor_tensor(
                        out=u_t[:].rearrange("p (h f) -> p h f", h=8),
                        in0=u_f[:, 0:256].rearrange("p (h f) -> p h f", h=8),
                        in1=s_rec[:].rearrange("p (h o) -> p h o", o=1).to_broadcast([128, 8, 32]),
                        op=mybir.AluOpType.mult)
                    # elu(x) = exp(-relu(-x)) - 1 + relu(x)
                    m0 = npool.tile([128, 256], f32, tag="m0")
                    nc.scalar.activation(out=m0[:], in_=u_t[:], func=RELUF, scale=-1.0)
                    e0 = npool.tile([128, 256], f32, tag="e0")
                    nc.scalar.activation(out=e0[:], in_=m0[:], func=EXPF, scale=-1.0)
                    r0 = npool.tile([128, 256], f32, tag="r0")
                    nc.scalar.activation(out=r0[:], in_=u_t[:], func=RELUF)
                    ub = npool.tile([128, 256], bf16, tag="ub")
                    nc.vector.scalar_tensor_tensor(
                        out=ub[:], in0=e0[:], scalar=-1.0, in1=r0[:],
                        op0=mybir.AluOpType.add, op1=mybir.AluOpType.add)
                    uT_ps = pnode.tile([128, 2, 128], bf16, tag="nps")
                    for t2 in range(2):
                        nc.tensor.transpose(out=uT_ps[:, t2, :], in_=ub[:, t2 * 128:(t2 + 1) * 128],
                                            identity=ident[:])
                    uT = npool.tile([128, 2, 128], bf16, tag="uT")
                    nc.scalar.activation(out=uT[:], in_=uT_ps[:], func=COPYF)
                    if l == 0:
                        h_ps = pnode.tile([128, 2, 128], f32, tag="nps")
                        for mt in range(2):
                            for kt in range(2):
                                nc.tensor.matmul(out=h_ps[:, mt, :],
                                                 lhsT=w2_t[:, kt, mt * 128:(mt + 1) * 128],
                                                 rhs=uT[:, kt, :],
                                                 start=(kt == 0), stop=(kt == 1))
                        h_T = npool.tile([128, 2, 128], bf16, tag="hT")
                        nc.scalar.activation(out=h_T[:], in_=h_ps[:], func=COPYF)
                        hb_ps = pnode.tile([128, 2, 128], bf16, tag="nps")
                        for t2 in range(2):
                            nc.tensor.transpose(out=hb_ps[:, t2, :], in_=h_T[:, t2, :],
                                                identity=ident[:])
                        hb = npool.tile([128, 256], f32, tag="hb")
                        nc.scalar.activation(out=hb[:], in_=hb_ps[:].rearrange("p a b -> p (a b)"),
                                             func=COPYF)
                        emit_table_rows(1, h_T, hb[:], wi)
                    else:
                        emit_table_rows(2, uT, ub[:], wi)
                else:
                    s_rec = npool.tile([128, 1], f32, tag="srec3")
                    nc.vector.tensor_scalar_add(s_rec[:], u_f[:, 256:257], 1e-16)
                    nc.vector.reciprocal(out=s_rec[:], in_=s_rec[:])
                    u_t = npool.tile([128, 256], f32, tag="ut")
                    nc.vector.tensor_tensor(
                        out=u_t[:], in0=u_f[:, 0:256],
                        in1=s_rec[:].to_broadcast([128, 256]), op=mybir.AluOpType.mult)
                    ub = npool.tile([128, 256], bf16, tag="ub")
                    nc.scalar.activation(out=ub[:], in_=u_t[:], func=COPYF)
                    uT_ps = pnode.tile([128, 2, 128], bf16, tag="nps")
                    for t2 in range(2):
                        nc.tensor.transpose(out=uT_ps[:, t2, :], in_=ub[:, t2 * 128:(t2 + 1) * 128],
                                            identity=ident[:])
                    uT = npool.tile([128, 2, 128], bf16, tag="uT")
                    nc.scalar.activation(out=uT[:], in_=uT_ps[:], func=COPYF)
                    o_ps = pnode.tile([N_CLASSES, 128], f32, tag="nps")
                    for kt in range(2):
                        nc.tensor.matmul(out=o_ps[:], lhsT=wo_t[:, kt, :],
                                         rhs=uT[:, kt, :], start=(kt == 0), stop=(kt == 1))
                    ob = npool.tile([128, 128], bf16, tag="ob")
                    nc.vector.memset(ob[:], 0.0)
                    nc.scalar.activation(out=ob[0:N_CLASSES, :], in_=o_ps[:], func=COPYF)
                    on_ps = pnode.tile([128, 128], bf16, tag="nps")
                    nc.tensor.transpose(out=on_ps[:], in_=ob[:], identity=ident[:])
                    o_n = npool.tile([128, N_CLASSES], f32, tag="on")
                    nc.scalar.activation(out=o_n[:], in_=on_ps[:, 0:N_CLASSES], func=COPYF)
                    mx = npool.tile([128, 1], f32, tag="mx")
                    nc.vector.tensor_reduce(out=mx[:], in_=o_n[:], axis=mybir.AxisListType.X,
                                            op=mybir.AluOpType.max)
                    nc.vector.tensor_tensor(out=o_n[:], in0=o_n[:],
                                            in1=mx[:].to_broadcast([128, N_CLASSES]),
                                            op=mybir.AluOpType.subtract)
                    ex_o = npool.tile([128, N_CLASSES], f32, tag="exo")
                    nc.scalar.activation(out=ex_o[:], in_=o_n[:], func=EXPF)
                    sm = npool.tile([128, 1], f32, tag="sm")
                    nc.vector.tensor_reduce(out=sm[:], in_=ex_o[:], axis=mybir.AxisListType.X,
                                            op=mybir.AluOpType.add)
                    ln_t = npool.tile([128, 1], f32, tag="lnt")
                    nc.scalar.activation(out=ln_t[:], in_=sm[:], func=mybir.ActivationFunctionType.Ln)
                    res = npool.tile([128, N_CLASSES], f32, tag="res")
                    nc.vector.tensor_tensor(out=res[:], in0=o_n[:],
                                            in1=ln_t[:].to_broadcast([128, N_CLASSES]),
                                            op=mybir.AluOpType.subtract)
                    nc.sync.dma_start(out=outy[wi * 128:(wi + 1) * 128, :], in_=res[:])

                # half-wise partial AllGather of the next table, fired from
                # inside the HIGH pass so it overlaps remaining windows
                if l < 2 and wi in (HT - 1, W - 1):
                    h = 0 if wi == HT - 1 else 1
                    nc.gpsimd.collective_compute(
                        "AllGather", mybir.AluOpType.bypass,
                        replica_groups=[list(range(CORES))],
                        ins=[Tsl_h[h].opt()],
                        outs=[Tfull[l + 1][h][:, :]])

    nc.compile()
    return nc


_CACHE = {}


def _make_in_maps(ins, per_core, n16s):
    CW, CMAX, off, soff, coff = _layout(n16s)
    x = np.asarray(ins["x"], np.float32)
    W1n = np.asarray(ins["W1"], np.float32)
    W2n = np.asarray(ins["W2"], np.float32)
    Won = np.asarray(ins["Wo"], np.float32)
    al1n, ar1n = np.asarray(ins["al1"], np.float32), np.asarray(ins["ar1"], np.float32)
    al2n, ar2n = np.asarray(ins["al2"], np.float32), np.asarray(ins["ar2"], np.float32)
    alon, aron = np.asarray(ins["alo"], np.float32), np.asarray(ins["aro"], np.float32)

    w1_h = W1n.astype(BF16)
    w2_h = np.ascontiguousarray(
        W2n.reshape(2, 128, 256).transpose(1, 0, 2)).astype(BF16)
    wo_h = np.ascontiguousarray(
        Won.reshape(2, 128, N_CLASSES).transpose(1, 0, 2)).astype(BF16)
    alar = np.zeros((3, 256, 16), np.float32)
    alar[0] = _alar_block(al1n, ar1n, 256)
    alar[1] = _alar_block(al2n, ar2n, 256)
    alar[2][:, 0] = Won @ alon[0]
    alar[2][:, 8] = Won @ aron[0]
    alar_h = np.ascontiguousarray(
        alar.reshape(3, 2, 128, 16).transpose(2, 0, 1, 3)).astype(BF16)

    mneg = np.zeros((1, SP), np.float32)
    mneg[0, SPR:] = -30000.0
    mneg_h = mneg.astype(BF16)

    in_maps = []
    for c in range(CORES):
        xs = np.zeros((SP, IN_F), np.float32)
        xs[:SPR] = x[c * SPR:(c + 1) * SPR]
        ed = per_core[c]
        gidx = [np.zeros((128, int(off[s][-1])), np.int16) for s in (0, 1)]
        selS = [np.zeros((128, int(soff[s][-1])), BF16) for s in (0, 1)]
        dcol = [np.full((128, int(coff[s][-1])), 400, BF16) for s in (0, 1)]
        for wi in range(W):
            for s in (0, 1):
                g, dl = ed[wi][s]
                n = len(g)
                n16 = int(n16s[wi][s])
                C = CW[wi][s]
                v = np.zeros(n16, np.int16)
                v[:n] = g
                gidx[s][:, int(off[s][wi]):int(off[s][wi]) + n16 // 16] = _wrap16(v)
                sS = np.zeros((128, C * 128), BF16)
                sS[dl, np.arange(n)] = 1
                selS[s][:, int(soff[s][wi]):int(soff[s][wi]) + C * 128] = sS
                dv = np.full(C * 128, 400.0, np.float32)
                dv[:n] = dl
                dcol[s][:, int(coff[s][wi]):int(coff[s][wi]) + C] = (
                    dv.reshape(C, 128).T.astype(BF16))
        in_maps.append(dict(
            xsl=xs, gidx0=gidx[0], gidx1=gidx[1],
            selS0=selS[0], selS1=selS[1], dcol0=dcol[0], dcol1=dcol[1],
            mneg=mneg_h, w1=w1_h, w2=w2_h, wo=wo_h, alar=alar_h))
    return in_maps


def kernel(x, src, dst, W1, al1, ar1, W2, al2, ar2, Wo, alo, aro):
    src = np.asarray(src, np.int32)
    dst = np.asarray(dst, np.int32)

    per_core, counts = _host_prep(src, dst)
    n16s = np.max(counts, axis=0)  # [W][2] max over cores

    key = tuple(n16s.flatten().tolist())
    if key not in _CACHE:
        _CACHE[key] = _build_program(n16s)
    nc = _CACHE[key]

    in_maps = _make_in_maps(
        dict(x=x, W1=W1, al1=al1, ar1=ar1, W2=W2, al2=al2, ar2=ar2,
             Wo=Wo, alo=alo, aro=aro), per_core, n16s)

    res = run_bass_kernel_spmd(nc, in_maps, list(range(CORES)))
    out = np.concatenate([res.results[c]["outy"][:SPR] for c in range(CORES)], axis=0)
    return out.astype(np.float32)


# revision 12
# speedup vs baseline: 1.0035x; 1.0035x over previous
"""3-layer GAT on Trainium2, 8 NeuronCores — v2.

Strategy (dst-sharded, replicated tables):
- Nodes are remapped into 8 slices of 6656 rows (6250 real + pad); each core
  owns one slice of destination nodes and all edges pointing into it.
- Per layer, every core builds its slice of a node table with 512-byte rows
  [h(256 fp8) | el(8 f32 = 32B) | pad]; an AllGather (Shared outputs)
  replicates the full 53248-row table to every core. er values for the
  core's own dst nodes stay in SBUF (erstore), never in the table.
- Edge phase per layer runs in TWO passes over the 49 dst windows: pass LOW
  aggregates the edges whose src lives in the first table half, stashing
  partial sums in SBUF; pass HIGH adds the rest, normalizes (softmax
  denominator after aggregation: alpha = ex/sum(ex) is scale invariant,
  |logits| < 10 so no segment-max) and runs the node phase. The half-0
  AllGather of the next layer's table (fired mid HIGH pass) then overlaps
  the next layer's LOW pass.
- Per (window, half): one dma_gather with exact (16-padded) edge counts;
  selT one-hots are generated on-chip (iota + is_equal against the dst-lane
  stream); selS (dst-major, needed to expand er to edges) streams from HBM;
  e = el_src + er_dst, leaky-relu + exp run edge-wise; one-hot matmuls
  segment-sum ex*h and ex into PSUM per dst.
- Layer 3 commutes the output projection with aggregation:
  sum(ex*h2) @ Wo == sum(ex*(h2@Wo)), with el3 = h2 @ (Wo@alo^T).
"""
import numpy as np
import ml_dtypes
from contextlib import ExitStack

import concourse.bass as bass
import concourse.tile as tile
from concourse import bacc, mybir
from concourse.bass_utils import run_bass_kernel_spmd
from concourse.masks import make_identity

BF16 = ml_dtypes.bfloat16
FP8 = ml_dtypes.float8_e4m3

N_NODES = 50000
IN_F = 128
N_CLASSES = 40
CORES = 8
SPR = 6250          # real dst nodes per core
SP = 6656           # slice rows per core (52 * 128)
NT = SP * CORES     # 53248 padded table rows
W = 49              # dst windows per core (ceil(6250/128))
ROWB = 512          # table row bytes (fp8 elems)
NTILE = SP // 128   # 52 node tiles per slice
EXPF = mybir.ActivationFunctionType.Exp
PRELU = mybir.ActivationFunctionType.Prelu
RELUF = mybir.ActivationFunctionType.Relu
COPYF = mybir.ActivationFunctionType.Copy

HT = NTILE // 2      # 26 tiles per collective half
HROWS = HT * 128     # 3328 slice rows per half
HBLK = HROWS * CORES # 26624 table rows per half; also the gather split
                     # boundary (aligns gather deps with one AllGather, and
                     # 26624 < 32768 keeps int16 indices valid)


def _remap(n):
    """Global table row for node n, laid out (half, core, tile, row) so that
    half-wise partial AllGathers are contiguous in both the slice and the
    full table (AllGather concatenates per-core inputs)."""
    c = n // SPR
    r = n % SPR
    t = r // 128
    h = t // HT
    return h * HBLK + c * HROWS + (t % HT) * 128 + (r % 128)


def _wrap16(vals):
    """dma_gather index layout: flat idx i -> [i%16, i//16], replicated to
    all 8 groups of 16 partitions. len(vals) must be a multiple of 16."""
    n = len(vals)
    blk = np.zeros((16, n // 16), np.int16)
    blk[np.arange(n) % 16, np.arange(n) // 16] = vals
    return np.tile(blk, (8, 1))


def _host_prep(src, dst):
    """Group edges by (dst core, dst window, table half). Returns per-core
    lists of (gather idx, dst lane) arrays indexed [W][2], plus the
    32-padded counts (32 keeps the chunk-tail memset partition-aligned)."""
    gsrc = _remap(src.astype(np.int64))
    d64 = dst.astype(np.int64)
    core = d64 // SPR
    ld = d64 % SPR
    w = ld >> 7
    dstl = (ld & 127).astype(np.int64)
    hi = (gsrc >= HBLK).astype(np.int64)

    key = (core * W + w) * 2 + hi
    order = np.argsort(key, kind="stable")
    gsrc_s, dstl_s, key_s = gsrc[order], dstl[order], key[order]

    per_core = []
    counts = np.zeros((CORES, W, 2), np.int64)
    for c in range(CORES):
        ed = [[None, None] for _ in range(W)]
        for wi in range(W):
            for s in (0, 1):
                k = (c * W + wi) * 2 + s
                lo = np.searchsorted(key_s, k)
                hiend = np.searchsorted(key_s, k + 1)
                g = (gsrc_s[lo:hiend] - (HBLK if s else 0)).astype(np.int16)
                dl = dstl_s[lo:hiend].astype(np.int16)
                ed[wi][s] = (g, dl)
                counts[c, wi, s] = max(((len(g) + 31) // 32) * 32, 32)
        per_core.append(ed)
    return per_core, counts


def _alar_block(al, ar, fout):
    """[fout, 16]: col j (<8) extracts el head j, col j+8 er head j."""
    H, F = al.shape
    m = np.zeros((fout, 16), np.float32)
    for j in range(H):
        m[j * F:(j + 1) * F, j] = al[j]
        m[j * F:(j + 1) * F, j + 8] = ar[j]
    return m


def _layout(n16s):
    """Shared program/data layout derived from the per-(window,half) counts
    (max over cores). Returns chunk counts and stream offsets."""
    CW = [[int((n16s[wi][s] + 127) // 128) for s in (0, 1)] for wi in range(W)]
    CMAX = max(max(r) for r in CW)
    off = [np.concatenate([[0], np.cumsum([int(n16s[wi][s]) // 16 for wi in range(W)])])
           for s in (0, 1)]
    soff = [np.concatenate([[0], np.cumsum([CW[wi][s] * 128 for wi in range(W)])])
            for s in (0, 1)]
    coff = [np.concatenate([[0], np.cumsum([CW[wi][s] for wi in range(W)])])
            for s in (0, 1)]
    return CW, CMAX, off, soff, coff


def _build_program(n16s):
    """n16s: [W][2] 16-padded edge counts (max over cores; all cores run
    identical shapes, shorter cores pad with idx 0 / lane 400)."""
    CW, CMAX, off, soff, coff = _layout(n16s)

    nc = bacc.Bacc("TRN2", target_bir_lowering=False, debug=False, num_devices=CORES)
    f32, bf16, i16 = mybir.dt.float32, mybir.dt.bfloat16, mybir.dt.int16
    fp8 = mybir.dt.float8e4

    xsl = nc.declare_dram_parameter("xsl", [SP, IN_F], f32, isOutput=False)
    gidx_d = [nc.declare_dram_parameter(f"gidx{s}", [128, int(off[s][-1])], i16,
                                        isOutput=False) for s in (0, 1)]
    selS_d = [nc.declare_dram_parameter(f"selS{s}", [128, int(soff[s][-1])], bf16,
                                        isOutput=False) for s in (0, 1)]
    dcol_d = [nc.declare_dram_parameter(f"dcol{s}", [128, int(coff[s][-1])], bf16,
                                        isOutput=False) for s in (0, 1)]
    mneg_d = nc.declare_dram_parameter("mneg", [1, SP], bf16, isOutput=False)
    w1_d = nc.declare_dram_parameter("w1", [IN_F, 256], bf16, isOutput=False)
    w2_d = nc.declare_dram_parameter("w2", [128, 2, 256], bf16, isOutput=False)
    wo_d = nc.declare_dram_parameter("wo", [128, 2, N_CLASSES], bf16, isOutput=False)
    alar_d = nc.declare_dram_parameter("alar", [128, 3, 2, 16], bf16, isOutput=False)
    outy = nc.declare_dram_parameter("outy", [SP, N_CLASSES], f32, isOutput=True)

    with ExitStack() as ctx:
        tc = ctx.enter_context(tile.TileContext(nc))
        const = ctx.enter_context(tc.tile_pool(name="const", bufs=1))
        dram = ctx.enter_context(tc.tile_pool(name="dram", bufs=1, space="DRAM"))
        gpool = ctx.enter_context(tc.tile_pool(name="gpool", bufs=2))
        spool = ctx.enter_context(tc.tile_pool(name="spool", bufs=2))
        npool = ctx.enter_context(tc.tile_pool(name="npool", bufs=2))
        pwin = ctx.enter_context(tc.tile_pool(name="pwin", bufs=2, space="PSUM"))
        pnode = ctx.enter_context(tc.tile_pool(name="pnode", bufs=2, space="PSUM"))

        Tsl_h = [dram.tile([HROWS, ROWB], fp8, name="tsl0"),
                 dram.tile([HROWS, ROWB], fp8, name="tsl1")]
        # one Shared tile per (layer, half): Shared DRAM allows exactly one
        # writing instruction, so no ping-pong reuse across layers
        Tfull = [[dram.tile([HBLK, ROWB], fp8, addr_space="Shared",
                            name=f"tf{l}{s}") for s in (0, 1)] for l in range(3)]

        def Tslice_rows(r0, r1):
            h = r0 // HROWS
            assert (r1 - 1) // HROWS == h
            return Tsl_h[h][r0 - h * HROWS:r1 - h * HROWS, :]

        gidx_t = []
        for s in (0, 1):
            t_ = const.tile([128, int(off[s][-1])], i16, name=f"gidx{s}t")
            nc.sync.dma_start(out=t_[:], in_=gidx_d[s][:, :])
            gidx_t.append(t_)
        dcol_t = []
        for s in (0, 1):
            t_ = const.tile([128, int(coff[s][-1])], bf16, name=f"dcol{s}t")
            nc.sync.dma_start(out=t_[:], in_=dcol_d[s][:, :])
            dcol_t.append(t_)
        mneg_t = const.tile([1, SP], bf16)
        nc.sync.dma_start(out=mneg_t[:], in_=mneg_d[:, :])
        w1_t = const.tile([IN_F, 256], bf16)
        nc.sync.dma_start(out=w1_t[:], in_=w1_d[:, :])
        w2_t = const.tile([128, 2, 256], bf16)
        nc.sync.dma_start(out=w2_t[:], in_=w2_d[:, :, :])
        wo_t = const.tile([128, 2, N_CLASSES], bf16)
        nc.sync.dma_start(out=wo_t[:], in_=wo_d[:, :, :])
        alar_t = const.tile([128, 3, 2, 16], bf16)
        nc.sync.dma_start(out=alar_t[:], in_=alar_d[:, :, :, :])
        ident = const.tile([128, 128], bf16)
        make_identity(nc, ident[:])
        ones16 = const.tile([1, 16], bf16)
        nc.vector.memset(ones16[:], 1.0)

        # iota grid for on-chip selT generation: iota_d[p, c, d] = d
        iota_d = const.tile([128, CMAX, 128], bf16)
        nc.gpsimd.iota(iota_d[:], pattern=[[0, CMAX], [1, 128]], base=0,
                       channel_multiplier=0, allow_small_or_imprecise_dtypes=True)

        # persistent per-layer stores
        erstore = [const.tile([128, NTILE, 8], bf16, name=f"ers{i}") for i in range(2)]
        stash = const.tile([128, W, 264], bf16)

        def emit_table_rows(l_next, h_T, h_node_src, w):
            """Assemble table row tile [h fp8 | el f32] for node rows
            [w*128,(w+1)*128) of layer l_next's table and DMA to Tslice.
            h_T: [128,2,128] bf16 feature-major; h_node_src: node-major
            h values [128, 256] (any float dtype)."""
            row_t = npool.tile([128, ROWB], fp8, tag="row")
            nc.vector.tensor_copy(out=row_t[:, 0:256], in_=h_node_src)
            el_ps = pnode.tile([16, 128], f32, tag="nps")
            for kt in range(2):
                nc.tensor.matmul(out=el_ps[:], lhsT=alar_t[:, l_next, kt, :],
                                 rhs=h_T[:, kt, :], start=(kt == 0), stop=False)
            nc.tensor.matmul(out=el_ps[:], lhsT=ones16[:],
                             rhs=mneg_t[:, w * 128:(w + 1) * 128], start=False, stop=True)
            elb = npool.tile([128, 128], bf16, tag="elb")
            nc.scalar.activation(out=elb[0:16, :], in_=el_ps[:], func=COPYF)
            elT_ps = pnode.tile([128, 128], bf16, tag="nps")
            nc.tensor.transpose(out=elT_ps[:], in_=elb[:], identity=ident[:])
            nc.scalar.activation(out=row_t[:, 256:288].bitcast(f32), in_=elT_ps[:, 0:8],
                                 func=COPYF)
            nc.scalar.activation(out=erstore[l_next % 2][:, w, :], in_=elT_ps[:, 8:16],
                                 func=COPYF)
            nc.sync.dma_start(out=Tslice_rows(w * 128, (w + 1) * 128), in_=row_t[:])

        # ---- P0: layer-1 table from x ----
        for t in range(NTILE):
            x_t = npool.tile([128, IN_F], f32, tag="xt")
            nc.sync.dma_start(out=x_t[:], in_=xsl[t * 128:(t + 1) * 128, :])
            xb = npool.tile([128, IN_F], bf16, tag="xb")
            nc.scalar.activation(out=xb[:], in_=x_t[:], func=COPYF)
            xT_ps = pnode.tile([128, 128], bf16, tag="nps")
            nc.tensor.transpose(out=xT_ps[:], in_=xb[:], identity=ident[:])
            xT = npool.tile([128, 128], bf16, tag="xT")
            nc.scalar.activation(out=xT[:], in_=xT_ps[:], func=COPYF)
            h_ps = pnode.tile([128, 2, 128], f32, tag="nps")
            for mt in range(2):
                nc.tensor.matmul(out=h_ps[:, mt, :], lhsT=w1_t[:, mt * 128:(mt + 1) * 128],
                                 rhs=xT[:], start=True, stop=True)
            h_T = npool.tile([128, 2, 128], bf16, tag="hT")
            nc.scalar.activation(out=h_T[:], in_=h_ps[:], func=COPYF)
            hb_ps = pnode.tile([128, 2, 128], bf16, tag="nps")
            for t2 in range(2):
                nc.tensor.transpose(out=hb_ps[:, t2, :], in_=h_T[:, t2, :], identity=ident[:])
            hb = npool.tile([128, 256], f32, tag="hb")
            nc.scalar.activation(out=hb[:], in_=hb_ps[:].rearrange("p a b -> p (a b)"),
                                 func=COPYF)
            emit_table_rows(0, h_T, hb[:], t)
            if t in (HT - 1, NTILE - 1):
                h = 0 if t == HT - 1 else 1
                nc.gpsimd.collective_compute(
                    "AllGather", mybir.AluOpType.bypass,
                    replica_groups=[list(range(CORES))],
                    ins=[Tsl_h[h].opt()],
                    outs=[Tfull[0][h][:, :]])

        # ---- 3 layers, each as LOW pass then HIGH pass over windows ----
        def window_half(l, wi, s):
            """Gather + aggregate one (window, half); returns agg PSUM tile."""
            C = CW[wi][s]
            n16 = int(n16s[wi][s])
            goff = int(off[s][wi])
            gwid = n16 // 16
            sof = int(soff[s][wi])
            cof = int(coff[s][wi])

            g_win = gpool.tile([128, C, ROWB], fp8, tag="gwin")
            nc.gpsimd.dma_gather(
                out_ap=g_win[:], in_ap=Tfull[l][s][:, :],
                idxs_ap=gidx_t[s][:, goff:goff + gwid],
                num_idxs=n16, num_idxs_reg=n16, elem_size=ROWB,
                single_packet=False)
            selS = spool.tile([128, C, 128], bf16, tag="selS")
            nc.sync.dma_start(out=selS[:].rearrange("p c d -> p (c d)"),
                              in_=selS_d[s][:, sof:sof + C * 128])
            selT = spool.tile([128, C, 128], bf16, tag="selT")
            nc.vector.tensor_tensor(
                out=selT[:], in0=iota_d[:, 0:C, :],
                in1=dcol_t[s][:, cof:cof + C].rearrange("p (c o) -> p c o", o=1)
                    .to_broadcast([128, C, 128]),
                op=mybir.AluOpType.is_equal)

            # er expansion to edges: pp[slot, 8]
            pp_ps = pwin.tile([128, C, 8], f32, tag="ppps")
            for c in range(C):
                nc.tensor.matmul(out=pp_ps[:, c, :], lhsT=selS[:, c, :],
                                 rhs=erstore[l % 2][:, wi, :], start=True, stop=True)
            # e = el + er ; leaky-relu (parametric_relu table) ; ex = exp(e)
            e_t = spool.tile([128, C, 8], f32, tag="et")
            nc.vector.tensor_tensor(
                out=e_t[:], in0=g_win[:, :, 256:288].bitcast(f32),
                in1=pp_ps[:], op=mybir.AluOpType.add)
            lk_t = spool.tile([128, C, 8], f32, tag="lkt")
            nc.scalar.activation(out=lk_t[:], in_=e_t[:], func=PRELU, alpha=0.2)
            ex_t = spool.tile([128, C, 8], bf16, tag="ext")
            nc.scalar.activation(out=ex_t[:], in_=lk_t[:], func=EXPF)

            rhs_w = spool.tile([128, C, 264], bf16, tag="rhsw")
            if l < 2:
                nc.vector.tensor_tensor(
                    out=rhs_w[:, :, 0:256].rearrange("p c (h f) -> p c h f", h=8),
                    in0=g_win[:, :, 0:256].rearrange("p c (h f) -> p c h f", h=8),
                    in1=ex_t[:].rearrange("p c (h o) -> p c h o", o=1)
                        .to_broadcast([128, C, 8, 32]),
                    op=mybir.AluOpType.mult)
            else:
                nc.vector.tensor_tensor(
                    out=rhs_w[:, :, 0:256],
                    in0=g_win[:, :, 0:256],
                    in1=ex_t[:, :, 0:1].to_broadcast([128, C, 256]),
                    op=mybir.AluOpType.mult)
            nc.scalar.activation(out=rhs_w[:, :, 256:264], in_=ex_t[:], func=COPYF)
            # chunk-tail slots (no gather, stale SBUF) could be NaN; zero
            # them in 32-partition pieces (engine APs with base partition
            # > 0 are limited to one 32-partition group)
            lane0 = n16 - (C - 1) * 128
            for b in range(lane0, 128, 32):
                nc.vector.memset(rhs_w[b:b + 32, C - 1, :], 0.0)

            agg_ps = pwin.tile([128, 264], f32, tag="aggps")
            for c in range(C):
                nc.tensor.matmul(out=agg_ps[:], lhsT=selT[:, c, :], rhs=rhs_w[:, c, :],
                                 start=(c == 0), stop=(c == C - 1))
            return agg_ps

        for l in range(3):
            # LOW pass
            for wi in range(W):
                agg_ps = window_half(l, wi, 0)
                nc.scalar.activation(out=stash[:, wi, :], in_=agg_ps[:], func=COPYF)
            # HIGH pass
            for wi in range(W):
                agg_ps = window_half(l, wi, 1)
                u_f = npool.tile([128, 264], f32, tag="uf")
                nc.vector.tensor_tensor(out=u_f[:], in0=agg_ps[:], in1=stash[:, wi, :],
                                        op=mybir.AluOpType.add)
                if l < 2:
                    s_rec = npool.tile([128, 8], f32, tag="srec")
                    nc.vector.tensor_scalar_add(s_rec[:], u_f[:, 256:264], 1e-16)
                    nc.vector.reciprocal(out=s_rec[:], in_=s_rec[:])
                    u_t = npool.tile([128, 256], f32, tag="ut")
                    nc.vector.tensor_tensor(
                        out=u_t[:].rearrange("p (h f) -> p h f", h=8),
                        in0=u_f[:, 0:256].rearrange("p (h f) -> p h f", h=8),
                        in1=s_rec[:].rearrange("p (h o) -> p h o", o=1).to_broadcast([128, 8, 32]),
                        op=mybir.AluOpType.mult)
                    # elu(x) = exp(-relu(-x)) - 1 + relu(x)
                    m0 = npool.tile([128, 256], f32, tag="m0")
                    nc.scalar.activation(out=m0[:], in_=u_t[:], func=RELUF, scale=-1.0)
                    e0 = npool.tile([128, 256], f32, tag="e0")
                    nc.scalar.activation(out=e0[:], in_=m0[:], func=EXPF, scale=-1.0)
                    r0 = npool.tile([128, 256], f32, tag="r0")
                    nc.scalar.activation(out=r0[:], in_=u_t[:], func=RELUF)
                    ub = npool.tile([128, 256], bf16, tag="ub")
                    nc.vector.scalar_tensor_tensor(
                        out=ub[:], in0=e0[:], scalar=-1.0, in1=r0[:],
                        op0=mybir.AluOpType.add, op1=mybir.AluOpType.add)
                    uT_ps = pnode.tile([128, 2, 128], bf16, tag="nps")
                    for t2 in range(2):
                        nc.tensor.transpose(out=uT_ps[:, t2, :], in_=ub[:, t2 * 128:(t2 + 1) * 128],
                                            identity=ident[:])
                    uT = npool.tile([128, 2, 128], bf16, tag="uT")
                    nc.scalar.activation(out=uT[:], in_=uT_ps[:], func=COPYF)
                    if l == 0:
                        h_ps = pnode.tile([128, 2, 128], f32, tag="nps")
                        for mt in range(2):
                            for kt in range(2):
                                nc.tensor.matmul(out=h_ps[:, mt, :],
                                                 lhsT=w2_t[:, kt, mt * 128:(mt + 1) * 128],
                                                 rhs=uT[:, kt, :],
                                                 start=(kt == 0), stop=(kt == 1))
                        h_T = npool.tile([128, 2, 128], bf16, tag="hT")
                        nc.scalar.activation(out=h_T[:], in_=h_ps[:], func=COPYF)
                        hb_ps = pnode.tile([128, 2, 128], bf16, tag="nps")
                        for t2 in range(2):
                            nc.tensor.transpose(out=hb_ps[:, t2, :], in_=h_T[:, t2, :],
                                                identity=ident[:])
                        hb = npool.tile([128, 256], f32, tag="hb")
                        nc.scalar.activation(out=hb[:], in_=hb_ps[:].rearrange("p a b -> p (a b)"),
                                             func=COPYF)
                        emit_table_rows(1, h_T, hb[:], wi)
                    else:
                        emit_table_rows(2, uT, ub[:], wi)
                else:
                    s_rec = npool.tile([128, 1], f32, tag="srec3")
                    nc.vector.tensor_scalar_add(s_rec[:], u_f[:, 256:257], 1e-16)
                    nc.vector.reciprocal(out=s_rec[:], in_=s_rec[:])
                    u_t = npool.tile([128, 256], f32, tag="ut")
                    nc.vector.tensor_tensor(
                        out=u_t[:], in0=u_f[:, 0:256],
                        in1=s_rec[:].to_broadcast([128, 256]), op=mybir.AluOpType.mult)
                    ub = npool.tile([128, 256], bf16, tag="ub")
                    nc.scalar.activation(out=ub[:], in_=u_t[:], func=COPYF)
                    uT_ps = pnode.tile([128, 2, 128], bf16, tag="nps")
                    for t2 in range(2):
                        nc.tensor.transpose(out=uT_ps[:, t2, :], in_=ub[:, t2 * 128:(t2 + 1) * 128],
                                            identity=ident[:])
                    uT = npool.tile([128, 2, 128], bf16, tag="uT")
                    nc.scalar.activation(out=uT[:], in_=uT_ps[:], func=COPYF)
                    o_ps = pnode.tile([N_CLASSES, 128], f32, tag="nps")
                    for kt in range(2):
                        nc.tensor.matmul(out=o_ps[:], lhsT=wo_t[:, kt, :],
                                         rhs=uT[:, kt, :], start=(kt == 0), stop=(kt == 1))
                    ob = npool.tile([128, 128], bf16, tag="ob")
                    nc.vector.memset(ob[:], 0.0)
                    nc.scalar.activation(out=ob[0:N_CLASSES, :], in_=o_ps[:], func=COPYF)
                    on_ps = pnode.tile([128, 128], bf16, tag="nps")
                    nc.tensor.transpose(out=on_ps[:], in_=ob[:], identity=ident[:])
                    o_n = npool.tile([128, N_CLASSES], f32, tag="on")
                    nc.scalar.activation(out=o_n[:], in_=on_ps[:, 0:N_CLASSES], func=COPYF)
                    mx = npool.tile([128, 1], f32, tag="mx")
                    nc.vector.tensor_reduce(out=mx[:], in_=o_n[:], axis=mybir.AxisListType.X,
                                            op=mybir.AluOpType.max)
                    nc.vector.tensor_tensor(out=o_n[:], in0=o_n[:],
                                            in1=mx[:].to_broadcast([128, N_CLASSES]),
                                            op=mybir.AluOpType.subtract)
                    ex_o = npool.tile([128, N_CLASSES], f32, tag="exo")
                    nc.scalar.activation(out=ex_o[:], in_=o_n[:], func=EXPF)
                    sm = npool.tile([128, 1], f32, tag="sm")
                    nc.vector.tensor_reduce(out=sm[:], in_=ex_o[:], axis=mybir.AxisListType.X,
                                            op=mybir.AluOpType.add)
                    ln_t = npool.tile([128, 1], f32, tag="lnt")
                    nc.scalar.activation(out=ln_t[:], in_=sm[:], func=mybir.ActivationFunctionType.Ln)
                    res = npool.tile([128, N_CLASSES], f32, tag="res")
                    nc.vector.tensor_tensor(out=res[:], in0=o_n[:],
                                            in1=ln_t[:].to_broadcast([128, N_CLASSES]),
                                            op=mybir.AluOpType.subtract)
                    nc.sync.dma_start(out=outy[wi * 128:(wi + 1) * 128, :], in_=res[:])

                # half-wise partial AllGather of the next table, fired from
                # inside the HIGH pass so it overlaps remaining windows
                if l < 2 and wi in (HT - 1, W - 1):
                    h = 0 if wi == HT - 1 else 1
                    nc.gpsimd.collective_compute(
                        "AllGather", mybir.AluOpType.bypass,
                        replica_groups=[list(range(CORES))],
                        ins=[Tsl_h[h].opt()],
                        outs=[Tfull[l + 1][h][:, :]])

    nc.compile()
    return nc


_CACHE = {}


def _make_in_maps(ins, per_core, n16s):
    CW, CMAX, off, soff, coff = _layout(n16s)
    x = np.asarray(ins["x"], np.float32)
    W1n = np.asarray(ins["W1"], np.float32)
    W2n = np.asarray(ins["W2"], np.float32)
    Won = np.asarray(ins["Wo"], np.float32)
    al1n, ar1n = np.asarray(ins["al1"], np.float32), np.asarray(ins["ar1"], np.float32)
    al2n, ar2n = np.asarray(ins["al2"], np.float32), np.asarray(ins["ar2"], np.float32)
    alon, aron = np.asarray(ins["alo"], np.float32), np.asarray(ins["aro"], np.float32)

    w1_h = W1n.astype(BF16)
    w2_h = np.ascontiguousarray(
        W2n.reshape(2, 128, 256).transpose(1, 0, 2)).astype(BF16)
    wo_h = np.ascontiguousarray(
        Won.reshape(2, 128, N_CLASSES).transpose(1, 0, 2)).astype(BF16)
    alar = np.zeros((3, 256, 16), np.float32)
    alar[0] = _alar_block(al1n, ar1n, 256)
    alar[1] = _alar_block(al2n, ar2n, 256)
    alar[2][:, 0] = Won @ alon[0]
    alar[2][:, 8] = Won @ aron[0]
    alar_h = np.ascontiguousarray(
        alar.reshape(3, 2, 128, 16).transpose(2, 0, 1, 3)).astype(BF16)

    mneg = np.zeros((1, SP), np.float32)
    mneg[0, SPR:] = -30000.0
    mneg_h = mneg.astype(BF16)

    in_maps = []
    for c in range(CORES):
        xs = np.zeros((SP, IN_F), np.float32)
        xs[:SPR] = x[c * SPR:(c + 1) * SPR]
        ed = per_core[c]
        gidx = [np.zeros((128, int(off[s][-1])), np.int16) for s in (0, 1)]
        selS = [np.zeros((128, int(soff[s][-1])), BF16) for s in (0, 1)]
        dcol = [np.full((128, int(coff[s][-1])), 400, BF16) for s in (0, 1)]
        for wi in range(W):
            for s in (0, 1):
                g, dl = ed[wi][s]
                n = len(g)
                n16 = int(n16s[wi][s])
                C = CW[wi][s]
                v = np.zeros(n16, np.int16)
                v[:n] = g
                gidx[s][:, int(off[s][wi]):int(off[s][wi]) + n16 // 16] = _wrap16(v)
                sS = np.zeros((128, C * 128), BF16)
                sS[dl, np.arange(n)] = 1
                selS[s][:, int(soff[s][wi]):int(soff[s][wi]) + C * 128] = sS
                dv = np.full(C * 128, 400.0, np.float32)
                dv[:n] = dl
                dcol[s][:, int(coff[s][wi]):int(coff[s][wi]) + C] = (
                    dv.reshape(C, 128).T.astype(BF16))
        in_maps.append(dict(
            xsl=xs, gidx0=gidx[0], gidx1=gidx[1],
            selS0=selS[0], selS1=selS[1], dcol0=dcol[0], dcol1=dcol[1],
            mneg=mneg_h, w1=w1_h, w2=w2_h, wo=wo_h, alar=alar_h))
    return in_maps


def kernel(x, src, dst, W1, al1, ar1, W2, al2, ar2, Wo, alo, aro):
    src = np.asarray(src, np.int32)
    dst = np.asarray(dst, np.int32)

    per_core, counts = _host_prep(src, dst)
    n16s = np.max(counts, axis=0)  # [W][2] max over cores

    key = tuple(n16s.flatten().tolist())
    if key not in _CACHE:
        _CACHE[key] = _build_program(n16s)
    nc = _CACHE[key]

    in_maps = _make_in_maps(
        dict(x=x, W1=W1, al1=al1, ar1=ar1, W2=W2, al2=al2, ar2=ar2,
             Wo=Wo, alo=alo, aro=aro), per_core, n16s)

    res = run_bass_kernel_spmd(nc, in_maps, list(range(CORES)))
    out = np.concatenate([res.results[c]["outy"][:SPR] for c in range(CORES)], axis=0)
    return out.astype(np.float32)


# revision 13
# speedup vs baseline: 1.1416x; 1.1376x over previous
"""3-layer GAT on Trainium2, 8 NeuronCores — v2.

Strategy (dst-sharded, replicated tables):
- Nodes are remapped into 8 slices of 6656 rows (6250 real + pad); each core
  owns one slice of destination nodes and all edges pointing into it.
- Per layer, every core builds its slice of a node table with 512-byte rows
  [h(256 fp8) | el(8 f32 = 32B) | pad]; an AllGather (Shared outputs)
  replicates the full 53248-row table to every core. er values for the
  core's own dst nodes stay in SBUF (erstore), never in the table.
- Edge phase per layer runs in TWO passes over the 49 dst windows: pass LOW
  aggregates the edges whose src lives in the first table half, stashing
  partial sums in SBUF; pass HIGH adds the rest, normalizes (softmax
  denominator after aggregation: alpha = ex/sum(ex) is scale invariant,
  |logits| < 10 so no segment-max) and runs the node phase. The half-0
  AllGather of the next layer's table (fired mid HIGH pass) then overlaps
  the next layer's LOW pass.
- Per (window, half): one dma_gather with exact (16-padded) edge counts;
  selT one-hots are generated on-chip (iota + is_equal against the dst-lane
  stream); selS (dst-major, needed to expand er to edges) streams from HBM;
  e = el_src + er_dst, leaky-relu + exp run edge-wise; one-hot matmuls
  segment-sum ex*h and ex into PSUM per dst.
- Layer 3 commutes the output projection with aggregation:
  sum(ex*h2) @ Wo == sum(ex*(h2@Wo)), with el3 = h2 @ (Wo@alo^T).
"""
import numpy as np
import ml_dtypes
from contextlib import ExitStack

import concourse.bass as bass
import concourse.tile as tile
from concourse import bacc, mybir
from concourse.bass_utils import run_bass_kernel_spmd
from concourse.masks import make_identity

BF16 = ml_dtypes.bfloat16
FP8 = ml_dtypes.float8_e4m3

N_NODES = 50000
IN_F = 128
N_CLASSES = 40
CORES = 8
SPR = 6250          # real dst nodes per core
SP = 6656           # slice rows per core (52 * 128)
NT = SP * CORES     # 53248 padded table rows
W = 49              # dst windows per core (ceil(6250/128))
ROWB = 512          # table row bytes (fp8 elems)
NTILE = SP // 128   # 52 node tiles per slice
EXPF = mybir.ActivationFunctionType.Exp
PRELU = mybir.ActivationFunctionType.Prelu
RELUF = mybir.ActivationFunctionType.Relu
COPYF = mybir.ActivationFunctionType.Copy

HT = NTILE // 2      # 26 tiles per collective half
HROWS = HT * 128     # 3328 slice rows per half
HBLK = HROWS * CORES # 26624 table rows per half; also the gather split
                     # boundary (aligns gather deps with one AllGather, and
                     # 26624 < 32768 keeps int16 indices valid)


def _remap(n):
    """Global table row for node n, laid out (half, core, tile, row) so that
    half-wise partial AllGathers are contiguous in both the slice and the
    full table (AllGather concatenates per-core inputs)."""
    c = n // SPR
    r = n % SPR
    t = r // 128
    h = t // HT
    return h * HBLK + c * HROWS + (t % HT) * 128 + (r % 128)


def _wrap16(vals):
    """dma_gather index layout: flat idx i -> [i%16, i//16], replicated to
    all 8 groups of 16 partitions. len(vals) must be a multiple of 16."""
    n = len(vals)
    blk = np.zeros((16, n // 16), np.int16)
    blk[np.arange(n) % 16, np.arange(n) // 16] = vals
    return np.tile(blk, (8, 1))


def _host_prep(src, dst):
    """Group edges by (dst core, dst window, table half). Returns per-core
    lists of (gather idx, dst lane) arrays indexed [W][2], plus the
    32-padded counts (32 keeps the chunk-tail memset partition-aligned)."""
    gsrc = _remap(src.astype(np.int64))
    d64 = dst.astype(np.int64)
    core = d64 // SPR
    ld = d64 % SPR
    w = ld >> 7
    dstl = (ld & 127).astype(np.int64)
    hi = (gsrc >= HBLK).astype(np.int64)

    key = (core * W + w) * 2 + hi
    order = np.argsort(key, kind="stable")
    gsrc_s, dstl_s, key_s = gsrc[order], dstl[order], key[order]

    per_core = []
    counts = np.zeros((CORES, W, 2), np.int64)
    for c in range(CORES):
        ed = [[None, None] for _ in range(W)]
        for wi in range(W):
            for s in (0, 1):
                k = (c * W + wi) * 2 + s
                lo = np.searchsorted(key_s, k)
                hiend = np.searchsorted(key_s, k + 1)
                g = (gsrc_s[lo:hiend] - (HBLK if s else 0)).astype(np.int16)
                dl = dstl_s[lo:hiend].astype(np.int16)
                ed[wi][s] = (g, dl)
                counts[c, wi, s] = max(((len(g) + 31) // 32) * 32, 32)
        per_core.append(ed)
    return per_core, counts


def _alar_block(al, ar, fout):
    """[fout, 16]: col j (<8) extracts el head j, col j+8 er head j."""
    H, F = al.shape
    m = np.zeros((fout, 16), np.float32)
    for j in range(H):
        m[j * F:(j + 1) * F, j] = al[j]
        m[j * F:(j + 1) * F, j + 8] = ar[j]
    return m


def _layout(n16s):
    """Shared program/data layout derived from the per-(window,half) counts
    (max over cores). Returns chunk counts and stream offsets."""
    CW = [[int((n16s[wi][s] + 127) // 128) for s in (0, 1)] for wi in range(W)]
    CMAX = max(max(r) for r in CW)
    off = [np.concatenate([[0], np.cumsum([int(n16s[wi][s]) // 16 for wi in range(W)])])
           for s in (0, 1)]
    soff = [np.concatenate([[0], np.cumsum([CW[wi][s] * 128 for wi in range(W)])])
            for s in (0, 1)]
    coff = [np.concatenate([[0], np.cumsum([CW[wi][s] for wi in range(W)])])
            for s in (0, 1)]
    return CW, CMAX, off, soff, coff


def _build_program(n16s):
    """n16s: [W][2] 16-padded edge counts (max over cores; all cores run
    identical shapes, shorter cores pad with idx 0 / lane 400)."""
    CW, CMAX, off, soff, coff = _layout(n16s)

    nc = bacc.Bacc("TRN2", target_bir_lowering=False, debug=False, num_devices=CORES)
    f32, bf16, i16 = mybir.dt.float32, mybir.dt.bfloat16, mybir.dt.int16
    fp8 = mybir.dt.float8e4

    xsl = nc.declare_dram_parameter("xsl", [SP, IN_F], f32, isOutput=False)
    gidx_d = [nc.declare_dram_parameter(f"gidx{s}", [128, int(off[s][-1])], i16,
                                        isOutput=False) for s in (0, 1)]
    selS_d = [nc.declare_dram_parameter(f"selS{s}", [128, int(soff[s][-1])], bf16,
                                        isOutput=False) for s in (0, 1)]
    dcol_d = [nc.declare_dram_parameter(f"dcol{s}", [128, int(coff[s][-1])], bf16,
                                        isOutput=False) for s in (0, 1)]
    mneg_d = nc.declare_dram_parameter("mneg", [1, SP], bf16, isOutput=False)
    w1_d = nc.declare_dram_parameter("w1", [IN_F, 256], bf16, isOutput=False)
    w2_d = nc.declare_dram_parameter("w2", [128, 2, 256], bf16, isOutput=False)
    wo_d = nc.declare_dram_parameter("wo", [128, 2, N_CLASSES], bf16, isOutput=False)
    alar_d = nc.declare_dram_parameter("alar", [128, 3, 2, 16], bf16, isOutput=False)
    outy = nc.declare_dram_parameter("outy", [SP, N_CLASSES], f32, isOutput=True)

    with ExitStack() as ctx:
        tc = ctx.enter_context(tile.TileContext(nc))
        const = ctx.enter_context(tc.tile_pool(name="const", bufs=1))
        dram = ctx.enter_context(tc.tile_pool(name="dram", bufs=1, space="DRAM"))
        gpool = ctx.enter_context(tc.tile_pool(name="gpool", bufs=2))
        spool = ctx.enter_context(tc.tile_pool(name="spool", bufs=2))
        npool = ctx.enter_context(tc.tile_pool(name="npool", bufs=2))
        pwin = ctx.enter_context(tc.tile_pool(name="pwin", bufs=2, space="PSUM"))
        pnode = ctx.enter_context(tc.tile_pool(name="pnode", bufs=3, space="PSUM"))

        Tsl_h = [dram.tile([HROWS, ROWB], fp8, name="tsl0"),
                 dram.tile([HROWS, ROWB], fp8, name="tsl1")]
        # one Shared tile per (layer, half): Shared DRAM allows exactly one
        # writing instruction, so no ping-pong reuse across layers
        Tfull = [[dram.tile([HBLK, ROWB], fp8, addr_space="Shared",
                            name=f"tf{l}{s}") for s in (0, 1)] for l in range(3)]

        def Tslice_rows(r0, r1):
            h = r0 // HROWS
            assert (r1 - 1) // HROWS == h
            return Tsl_h[h][r0 - h * HROWS:r1 - h * HROWS, :]

        gidx_t = []
        for s in (0, 1):
            t_ = const.tile([128, int(off[s][-1])], i16, name=f"gidx{s}t")
            nc.sync.dma_start(out=t_[:], in_=gidx_d[s][:, :])
            gidx_t.append(t_)
        dcol_t = []
        for s in (0, 1):
            t_ = const.tile([128, int(coff[s][-1])], bf16, name=f"dcol{s}t")
            nc.sync.dma_start(out=t_[:], in_=dcol_d[s][:, :])
            dcol_t.append(t_)
        mneg_t = const.tile([1, SP], bf16)
        nc.sync.dma_start(out=mneg_t[:], in_=mneg_d[:, :])
        w1_t = const.tile([IN_F, 256], bf16)
        nc.sync.dma_start(out=w1_t[:], in_=w1_d[:, :])
        w2_t = const.tile([128, 2, 256], bf16)
        nc.sync.dma_start(out=w2_t[:], in_=w2_d[:, :, :])
        wo_t = const.tile([128, 2, N_CLASSES], bf16)
        nc.sync.dma_start(out=wo_t[:], in_=wo_d[:, :, :])
        alar_t = const.tile([128, 3, 2, 16], bf16)
        nc.sync.dma_start(out=alar_t[:], in_=alar_d[:, :, :, :])
        ident = const.tile([128, 128], bf16)
        make_identity(nc, ident[:])
        ones16 = const.tile([1, 16], bf16)
        nc.vector.memset(ones16[:], 1.0)

        # iota grid for on-chip selT generation: iota_d[p, c, d] = d
        iota_d = const.tile([128, CMAX, 128], bf16)
        nc.gpsimd.iota(iota_d[:], pattern=[[0, CMAX], [1, 128]], base=0,
                       channel_multiplier=0, allow_small_or_imprecise_dtypes=True)

        # persistent per-layer stores
        erstore = [const.tile([128, NTILE, 8], bf16, name=f"ers{i}") for i in range(2)]
        stash = const.tile([128, W, 264], bf16)

        def emit_table_rows(l_next, h_T, h_node_src, w):
            """Assemble table row tile [h fp8 | el f32] for node rows
            [w*128,(w+1)*128) of layer l_next's table and DMA to Tslice.
            h_T: [128,2,128] bf16 feature-major; h_node_src: node-major
            h values [128, 256] (any float dtype)."""
            row_t = npool.tile([128, ROWB], fp8, tag="row")
            nc.vector.tensor_copy(out=row_t[:, 0:256], in_=h_node_src)
            el_ps = pnode.tile([16, 128], f32, tag="nps")
            for kt in range(2):
                nc.tensor.matmul(out=el_ps[:], lhsT=alar_t[:, l_next, kt, :],
                                 rhs=h_T[:, kt, :], start=(kt == 0), stop=False)
            nc.tensor.matmul(out=el_ps[:], lhsT=ones16[:],
                             rhs=mneg_t[:, w * 128:(w + 1) * 128], start=False, stop=True)
            elb = npool.tile([128, 128], bf16, tag="elb")
            nc.scalar.activation(out=elb[0:16, :], in_=el_ps[:], func=COPYF)
            elT_ps = pnode.tile([128, 128], bf16, tag="nps")
            nc.tensor.transpose(out=elT_ps[:], in_=elb[:], identity=ident[:])
            nc.scalar.activation(out=row_t[:, 256:288].bitcast(f32), in_=elT_ps[:, 0:8],
                                 func=COPYF)
            nc.scalar.activation(out=erstore[l_next % 2][:, w, :], in_=elT_ps[:, 8:16],
                                 func=COPYF)
            nc.sync.dma_start(out=Tslice_rows(w * 128, (w + 1) * 128), in_=row_t[:])

        # ---- P0: layer-1 table from x ----
        for t in range(NTILE):
            x_t = npool.tile([128, IN_F], f32, tag="xt")
            nc.sync.dma_start(out=x_t[:], in_=xsl[t * 128:(t + 1) * 128, :])
            xb = npool.tile([128, IN_F], bf16, tag="xb")
            nc.scalar.activation(out=xb[:], in_=x_t[:], func=COPYF)
            xT_ps = pnode.tile([128, 128], bf16, tag="nps")
            nc.tensor.transpose(out=xT_ps[:], in_=xb[:], identity=ident[:])
            xT = npool.tile([128, 128], bf16, tag="xT")
            nc.scalar.activation(out=xT[:], in_=xT_ps[:], func=COPYF)
            h_ps = pnode.tile([128, 2, 128], f32, tag="nps")
            for mt in range(2):
                nc.tensor.matmul(out=h_ps[:, mt, :], lhsT=w1_t[:, mt * 128:(mt + 1) * 128],
                                 rhs=xT[:], start=True, stop=True)
            h_T = npool.tile([128, 2, 128], bf16, tag="hT")
            nc.scalar.activation(out=h_T[:], in_=h_ps[:], func=COPYF)
            hb_ps = pnode.tile([128, 2, 128], bf16, tag="nps")
            for t2 in range(2):
                nc.tensor.transpose(out=hb_ps[:, t2, :], in_=h_T[:, t2, :], identity=ident[:])
            hb = npool.tile([128, 256], f32, tag="hb")
            nc.scalar.activation(out=hb[:], in_=hb_ps[:].rearrange("p a b -> p (a b)"),
                                 func=COPYF)
            emit_table_rows(0, h_T, hb[:], t)
            if t in (HT - 1, NTILE - 1):
                h = 0 if t == HT - 1 else 1
                nc.gpsimd.collective_compute(
                    "AllGather", mybir.AluOpType.bypass,
                    replica_groups=[list(range(CORES))],
                    ins=[Tsl_h[h].opt()],
                    outs=[Tfull[0][h][:, :]])

        # ---- 3 layers, each as LOW pass then HIGH pass over windows ----
        def window_half(l, wi, s):
            """Gather + aggregate one (window, half); returns agg PSUM tile."""
            C = CW[wi][s]
            n16 = int(n16s[wi][s])
            goff = int(off[s][wi])
            gwid = n16 // 16
            sof = int(soff[s][wi])
            cof = int(coff[s][wi])

            g_win = gpool.tile([128, C, ROWB], fp8, tag="gwin")
            nc.gpsimd.dma_gather(
                out_ap=g_win[:], in_ap=Tfull[l][s][:, :],
                idxs_ap=gidx_t[s][:, goff:goff + gwid],
                num_idxs=n16, num_idxs_reg=n16, elem_size=ROWB,
                single_packet=False)
            selS = spool.tile([128, C, 128], bf16, tag="selS")
            nc.sync.dma_start(out=selS[:].rearrange("p c d -> p (c d)"),
                              in_=selS_d[s][:, sof:sof + C * 128])
            selT = spool.tile([128, C, 128], bf16, tag="selT")
            nc.vector.tensor_tensor(
                out=selT[:], in0=iota_d[:, 0:C, :],
                in1=dcol_t[s][:, cof:cof + C].rearrange("p (c o) -> p c o", o=1)
                    .to_broadcast([128, C, 128]),
                op=mybir.AluOpType.is_equal)

            # er expansion to edges: pp[slot, 8]
            pp_ps = pwin.tile([128, C, 8], f32, tag="ppps")
            for c in range(C):
                nc.tensor.matmul(out=pp_ps[:, c, :], lhsT=selS[:, c, :],
                                 rhs=erstore[l % 2][:, wi, :], start=True, stop=True)
            # e = el + er ; leaky-relu (parametric_relu table) ; ex = exp(e)
            e_t = spool.tile([128, C, 8], f32, tag="et")
            nc.vector.tensor_tensor(
                out=e_t[:], in0=g_win[:, :, 256:288].bitcast(f32),
                in1=pp_ps[:], op=mybir.AluOpType.add)
            lk_t = spool.tile([128, C, 8], f32, tag="lkt")
            nc.scalar.activation(out=lk_t[:], in_=e_t[:], func=PRELU, alpha=0.2)
            ex_t = spool.tile([128, C, 8], bf16, tag="ext")
            nc.scalar.activation(out=ex_t[:], in_=lk_t[:], func=EXPF)

            rhs_w = spool.tile([128, C, 264], bf16, tag="rhsw")
            if l < 2:
                nc.vector.tensor_tensor(
                    out=rhs_w[:, :, 0:256].rearrange("p c (h f) -> p c h f", h=8),
                    in0=g_win[:, :, 0:256].rearrange("p c (h f) -> p c h f", h=8),
                    in1=ex_t[:].rearrange("p c (h o) -> p c h o", o=1)
                        .to_broadcast([128, C, 8, 32]),
                    op=mybir.AluOpType.mult)
            else:
                nc.vector.tensor_tensor(
                    out=rhs_w[:, :, 0:256],
                    in0=g_win[:, :, 0:256],
                    in1=ex_t[:, :, 0:1].to_broadcast([128, C, 256]),
                    op=mybir.AluOpType.mult)
            nc.scalar.activation(out=rhs_w[:, :, 256:264], in_=ex_t[:], func=COPYF)
            # chunk-tail slots (no gather, stale SBUF) could be NaN; zero
            # them in 32-partition pieces (engine APs with base partition
            # > 0 are limited to one 32-partition group)
            lane0 = n16 - (C - 1) * 128
            for b in range(lane0, 128, 32):
                nc.vector.memset(rhs_w[b:b + 32, C - 1, :], 0.0)

            agg_ps = pwin.tile([128, 264], f32, tag="aggps")
            for c in range(C):
                nc.tensor.matmul(out=agg_ps[:], lhsT=selT[:, c, :], rhs=rhs_w[:, c, :],
                                 start=(c == 0), stop=(c == C - 1))
            return agg_ps

        for l in range(3):
            # LOW pass
            for wi in range(W):
                agg_ps = window_half(l, wi, 0)
                nc.scalar.activation(out=stash[:, wi, :], in_=agg_ps[:], func=COPYF)
            # HIGH pass
            for wi in range(W):
                agg_ps = window_half(l, wi, 1)
                u_f = npool.tile([128, 264], f32, tag="uf")
                nc.vector.tensor_tensor(out=u_f[:], in0=agg_ps[:], in1=stash[:, wi, :],
                                        op=mybir.AluOpType.add)
                if l < 2:
                    s_rec = npool.tile([128, 8], f32, tag="srec")
                    nc.vector.tensor_scalar_add(s_rec[:], u_f[:, 256:264], 1e-16)
                    nc.vector.reciprocal(out=s_rec[:], in_=s_rec[:])
                    u_t = npool.tile([128, 256], f32, tag="ut")
                    nc.vector.tensor_tensor(
                        out=u_t[:].rearrange("p (h f) -> p h f", h=8),
                        in0=u_f[:, 0:256].rearrange("p (h f) -> p h f", h=8),
                        in1=s_rec[:].rearrange("p (h o) -> p h o", o=1).to_broadcast([128, 8, 32]),
                        op=mybir.AluOpType.mult)
                    # elu(x) = exp(-relu(-x)) - 1 + relu(x)
                    m0 = npool.tile([128, 256], f32, tag="m0")
                    nc.scalar.activation(out=m0[:], in_=u_t[:], func=RELUF, scale=-1.0)
                    e0 = npool.tile([128, 256], f32, tag="e0")
                    nc.scalar.activation(out=e0[:], in_=m0[:], func=EXPF, scale=-1.0)
                    r0 = npool.tile([128, 256], f32, tag="r0")
                    nc.scalar.activation(out=r0[:], in_=u_t[:], func=RELUF)
                    ub = npool.tile([128, 256], bf16, tag="ub")
                    nc.vector.scalar_tensor_tensor(
                        out=ub[:], in0=e0[:], scalar=-1.0, in1=r0[:],
                        op0=mybir.AluOpType.add, op1=mybir.AluOpType.add)
                    uT_ps = pnode.tile([128, 2, 128], bf16, tag="nps")
                    for t2 in range(2):
                        nc.tensor.transpose(out=uT_ps[:, t2, :], in_=ub[:, t2 * 128:(t2 + 1) * 128],
                                            identity=ident[:])
                    uT = npool.tile([128, 2, 128], bf16, tag="uT")
                    nc.scalar.activation(out=uT[:], in_=uT_ps[:], func=COPYF)
                    if l == 0:
                        h_ps = pnode.tile([128, 2, 128], f32, tag="nps")
                        for mt in range(2):
                            for kt in range(2):
                                nc.tensor.matmul(out=h_ps[:, mt, :],
                                                 lhsT=w2_t[:, kt, mt * 128:(mt + 1) * 128],
                                                 rhs=uT[:, kt, :],
                                                 start=(kt == 0), stop=(kt == 1))
                        h_T = npool.tile([128, 2, 128], bf16, tag="hT")
                        nc.scalar.activation(out=h_T[:], in_=h_ps[:], func=COPYF)
                        hb_ps = pnode.tile([128, 2, 128], bf16, tag="nps")
                        for t2 in range(2):
                            nc.tensor.transpose(out=hb_ps[:, t2, :], in_=h_T[:, t2, :],
                                                identity=ident[:])
                        hb = npool.tile([128, 256], f32, tag="hb")
                        nc.scalar.activation(out=hb[:], in_=hb_ps[:].rearrange("p a b -> p (a b)"),
                                             func=COPYF)
                        emit_table_rows(1, h_T, hb[:], wi)
                    else:
                        emit_table_rows(2, uT, ub[:], wi)
                else:
                    s_rec = npool.tile([128, 1], f32, tag="srec3")
                    nc.vector.tensor_scalar_add(s_rec[:], u_f[:, 256:257], 1e-16)
                    nc.vector.reciprocal(out=s_rec[:], in_=s_rec[:])
                    u_t = npool.tile([128, 256], f32, tag="ut")
                    nc.vector.tensor_tensor(
                        out=u_t[:], in0=u_f[:, 0:256],
                        in1=s_rec[:].to_broadcast([128, 256]), op=mybir.AluOpType.mult)
                    ub = npool.tile([128, 256], bf16, tag="ub")
                    nc.scalar.activation(out=ub[:], in_=u_t[:], func=COPYF)
                    uT_ps = pnode.tile([128, 2, 128], bf16, tag="nps")
                    for t2 in range(2):
                        nc.tensor.transpose(out=uT_ps[:, t2, :], in_=ub[:, t2 * 128:(t2 + 1) * 128],
                                            identity=ident[:])
                    uT = npool.tile([128, 2, 128], bf16, tag="uT")
                    nc.scalar.activation(out=uT[:], in_=uT_ps[:], func=COPYF)
                    o_ps = pnode.tile([N_CLASSES, 128], f32, tag="nps")
                    for kt in range(2):
                        nc.tensor.matmul(out=o_ps[:], lhsT=wo_t[:, kt, :],
                                         rhs=uT[:, kt, :], start=(kt == 0), stop=(kt == 1))
                    ob = npool.tile([128, 128], bf16, tag="ob")
                    nc.vector.memset(ob[:], 0.0)
                    nc.scalar.activation(out=ob[0:N_CLASSES, :], in_=o_ps[:], func=COPYF)
                    on_ps = pnode.tile([128, 128], bf16, tag="nps")
                    nc.tensor.transpose(out=on_ps[:], in_=ob[:], identity=ident[:])
                    o_n = npool.tile([128, N_CLASSES], f32, tag="on")
                    nc.scalar.activation(out=o_n[:], in_=on_ps[:, 0:N_CLASSES], func=COPYF)
                    mx = npool.tile([128, 1], f32, tag="mx")
                    nc.vector.tensor_reduce(out=mx[:], in_=o_n[:], axis=mybir.AxisListType.X,
                                            op=mybir.AluOpType.max)
                    nc.vector.tensor_tensor(out=o_n[:], in0=o_n[:],
                                            in1=mx[:].to_broadcast([128, N_CLASSES]),
                                            op=mybir.AluOpType.subtract)
                    ex_o = npool.tile([128, N_CLASSES], f32, tag="exo")
                    nc.scalar.activation(out=ex_o[:], in_=o_n[:], func=EXPF)
                    sm = npool.tile([128, 1], f32, tag="sm")
                    nc.vector.tensor_reduce(out=sm[:], in_=ex_o[:], axis=mybir.AxisListType.X,
                                            op=mybir.AluOpType.add)
                    ln_t = npool.tile([128, 1], f32, tag="lnt")
                    nc.scalar.activation(out=ln_t[:], in_=sm[:], func=mybir.ActivationFunctionType.Ln)
                    res = npool.tile([128, N_CLASSES], f32, tag="res")
                    nc.vector.tensor_tensor(out=res[:], in0=o_n[:],
                                            in1=ln_t[:].to_broadcast([128, N_CLASSES]),
                                            op=mybir.AluOpType.subtract)
                    nc.sync.dma_start(out=outy[wi * 128:(wi + 1) * 128, :], in_=res[:])

                # half-wise partial AllGather of the next table, fired from
                # inside the HIGH pass so it overlaps remaining windows
                if l < 2 and wi in (HT - 1, W - 1):
                    h = 0 if wi == HT - 1 else 1
                    nc.gpsimd.collective_compute(
                        "AllGather", mybir.AluOpType.bypass,
                        replica_groups=[list(range(CORES))],
                        ins=[Tsl_h[h].opt()],
                        outs=[Tfull[l + 1][h][:, :]])

    # Pin all activations to the one LUT set containing exp+ln+relu+
    # parametric_relu+copy, so the ACT engine never reloads tables
    # mid-stream (Exp<->Ln switching cost ~1.3us x 96 otherwise). Other
    # entries are blanked rather than removed to preserve the global
    # act_func_set_id indexing that walrus expects.
    import concourse.bacc as _bacc_mod
    _orig_tables = _bacc_mod.get_activation_tables
    _PIN = "natural_log_exp_and_others"

    def _pinned(arch):
        t = _orig_tables(arch)
        return {k: (v if k == _PIN else set()) for k, v in t.items()}

    _bacc_mod.get_activation_tables = _pinned
    try:
        nc.compile()
    finally:
        _bacc_mod.get_activation_tables = _orig_tables
    return nc


_CACHE = {}


def _make_in_maps(ins, per_core, n16s):
    CW, CMAX, off, soff, coff = _layout(n16s)
    x = np.asarray(ins["x"], np.float32)
    W1n = np.asarray(ins["W1"], np.float32)
    W2n = np.asarray(ins["W2"], np.float32)
    Won = np.asarray(ins["Wo"], np.float32)
    al1n, ar1n = np.asarray(ins["al1"], np.float32), np.asarray(ins["ar1"], np.float32)
    al2n, ar2n = np.asarray(ins["al2"], np.float32), np.asarray(ins["ar2"], np.float32)
    alon, aron = np.asarray(ins["alo"], np.float32), np.asarray(ins["aro"], np.float32)

    w1_h = W1n.astype(BF16)
    w2_h = np.ascontiguousarray(
        W2n.reshape(2, 128, 256).transpose(1, 0, 2)).astype(BF16)
    wo_h = np.ascontiguousarray(
        Won.reshape(2, 128, N_CLASSES).transpose(1, 0, 2)).astype(BF16)
    alar = np.zeros((3, 256, 16), np.float32)
    alar[0] = _alar_block(al1n, ar1n, 256)
    alar[1] = _alar_block(al2n, ar2n, 256)
    alar[2][:, 0] = Won @ alon[0]
    alar[2][:, 8] = Won @ aron[0]
    alar_h = np.ascontiguousarray(
        alar.reshape(3, 2, 128, 16).transpose(2, 0, 1, 3)).astype(BF16)

    mneg = np.zeros((1, SP), np.float32)
    mneg[0, SPR:] = -30000.0
    mneg_h = mneg.astype(BF16)

    in_maps = []
    for c in range(CORES):
        xs = np.zeros((SP, IN_F), np.float32)
        xs[:SPR] = x[c * SPR:(c + 1) * SPR]
        ed = per_core[c]
        gidx = [np.zeros((128, int(off[s][-1])), np.int16) for s in (0, 1)]
        selS = [np.zeros((128, int(soff[s][-1])), BF16) for s in (0, 1)]
        dcol = [np.full((128, int(coff[s][-1])), 400, BF16) for s in (0, 1)]
        for wi in range(W):
            for s in (0, 1):
                g, dl = ed[wi][s]
                n = len(g)
                n16 = int(n16s[wi][s])
                C = CW[wi][s]
                v = np.zeros(n16, np.int16)
                v[:n] = g
                gidx[s][:, int(off[s][wi]):int(off[s][wi]) + n16 // 16] = _wrap16(v)
                sS = np.zeros((128, C * 128), BF16)
                sS[dl, np.arange(n)] = 1
                selS[s][:, int(soff[s][wi]):int(soff[s][wi]) + C * 128] = sS
                dv = np.full(C * 128, 400.0, np.float32)
                dv[:n] = dl
                dcol[s][:, int(coff[s][wi]):int(coff[s][wi]) + C] = (
                    dv.reshape(C, 128).T.astype(BF16))
        in_maps.append(dict(
            xsl=xs, gidx0=gidx[0], gidx1=gidx[1],
            selS0=selS[0], selS1=selS[1], dcol0=dcol[0], dcol1=dcol[1],
            mneg=mneg_h, w1=w1_h, w2=w2_h, wo=wo_h, alar=alar_h))
    return in_maps


def kernel(x, src, dst, W1, al1, ar1, W2, al2, ar2, Wo, alo, aro):
    src = np.asarray(src, np.int32)
    dst = np.asarray(dst, np.int32)

    per_core, counts = _host_prep(src, dst)
    n16s = np.max(counts, axis=0)  # [W][2] max over cores

    key = tuple(n16s.flatten().tolist())
    if key not in _CACHE:
        _CACHE[key] = _build_program(n16s)
    nc = _CACHE[key]

    in_maps = _make_in_maps(
        dict(x=x, W1=W1, al1=al1, ar1=ar1, W2=W2, al2=al2, ar2=ar2,
             Wo=Wo, alo=alo, aro=aro), per_core, n16s)

    res = run_bass_kernel_spmd(nc, in_maps, list(range(CORES)))
    out = np.concatenate([res.results[c]["outy"][:SPR] for c in range(CORES)], axis=0)
    return out.astype(np.float32)
